# revision 13
# baseline (speedup 1.0000x reference)
"""Binary ResNet block (bireal) Trainium2 Bass kernel.

Data-parallel over 8 NeuronCores: batch 32 -> 4 images/core.
Layout: partition p = img*32 + channel (128 partitions).

Per core:
  sign(x) -> fp8 padded buffer B; conv = 5 DoubleRow fp8 matmuls/chunk
  (block-diag sign(w) weights, 2 taps per matmul) accumulating exact
  integer counts z in PSUM f32; z stored fp16 (exact, |z|<=288).
  BN stats via bn_stats/bn_aggr per chunk + [128,2] AllReduce across
  cores; per-channel affine a*z+b with the binarize scale folded in.
  inner = a1*z1 + x (+b1 folded into the Sign bias of the conv2 input);
  out = a2*z2 + inner + (b1+b2).
"""
import numpy as np

import concourse.bass as bass
import concourse.tile as tile
from concourse import bacc, mybir, bass_isa
from concourse.bass_utils import run_bass_kernel_spmd

F32 = mybir.dt.float32
F16 = mybir.dt.float16
F8 = mybir.dt.float8e4
AF = mybir.ActivationFunctionType
ALU = mybir.AluOpType

N_CORES = 8
C = 32
H = W = 160
IMG = 4            # images per core
P = 128            # IMG * C
NPIX = H * W       # 25600
Wp = W + 2         # 162 padded row
PADH = 164         # head margin elems in B
BODY = Wp * Wp     # 26244
BSZ = PADH + BODY + 256
EPS = 1e-5
NW = C * C * 9     # 9216 weight elems

# conv chunks: (padded row start, nrows); padded row r = real row r-1
CHUNKS = [(1 + 3 * k, 3) for k in range(53)] + [(160, 1)]
# tap t=(kh,kw): delta in padded flat index
DELTA = [(kh - 1) * Wp + (kw - 1) for kh in range(3) for kw in range(3)]

_CACHE = {}
DEBUG_NAMES = {}
USE_DP = True  # DoublePixel fp8 perf mode (HW only; CoreSim lacks it)


def _valid_ap(t, extra_off, nr, rstride):
    """[128, nr, 160] view at offset extra_off with row stride rstride."""
    return bass.AP(tensor=t.tensor, offset=t.offset + extra_off,
                   ap=[t.ap[0], [rstride, nr], [1, W]])


def _build():
    nc = bacc.Bacc("TRN2", target_bir_lowering=False, debug=False,
                   num_devices=N_CORES)
    x_in = nc.dram_tensor("x", [IMG, C, H, W], F32, kind="ExternalInput").ap()
    w_in = [nc.dram_tensor(f"w{j+1}", [NW, 1], F32, kind="ExternalInput").ap()
            for j in range(2)]
    g_in = [nc.dram_tensor(f"gamma{j+1}", [C], F32, kind="ExternalInput").ap()
            for j in range(2)]
    be_in = [nc.dram_tensor(f"beta{j+1}", [C], F32, kind="ExternalInput").ap()
             for j in range(2)]
    out = nc.dram_tensor("out", [IMG, C, H, W], F32, kind="ExternalOutput").ap()

    x2d = x_in.rearrange("a c h w -> (a c) (h w)")
    out2d = out.rearrange("a c h w -> (a c) (h w)")

    with tile.TileContext(nc) as tc:
        with tc.tile_pool(name="big", bufs=1) as big, \
             tc.tile_pool(name="small", bufs=1) as small, \
             tc.tile_pool(name="ps", bufs=8, space="PSUM") as ps, \
             tc.tile_pool(name="dram", bufs=1, space="DRAM") as dram:

            B = big.tile([P, BSZ], F8)
            xsb = big.tile([P, NPIX], F32)     # x, then inner (in place)
            z = big.tile([P, NPIX], F16)       # z1 then z2
            NCH = len(CHUNKS)
            stats = big.tile([P, NCH, 6], F32)  # bn_stats, reused per conv
            junk = big.tile([P, NCH], F32)      # product scratch

            def scol(i):
                return bass.AP(tensor=stats.tensor, offset=stats.offset + i,
                               ap=[stats.ap[0], [6, NCH]])

            DEBUG_NAMES.update(z=z.tensor.name, B=B.tensor.name,
                               xsb=xsb.tensor.name, stats=stats.tensor.name)

            # ---- B border zeros (once; sign passes only write valid cells)
            nc.gpsimd.memset(B[:, 0:PADH + Wp + 1], 0.0)
            m2 = bass.AP(tensor=B.tensor, offset=B.offset + PADH + Wp + W + 1,
                         ap=[B.ap[0], [Wp, H - 1], [1, 2]])
            nc.gpsimd.memset(m2, 0.0)
            nc.gpsimd.memset(B[:, PADH + H * Wp + W + 1:BSZ], 0.0)

            # ---- weight prep for both convs
            sfac, st_tiles, gsb, bsb = [], [], [], []
            for j in range(2):
                wflat = small.tile([C, 288], F32, name=f"wflat{j}")
                nc.sync.dma_start(
                    out=wflat[:],
                    in_=w_in[j].rearrange("(co k) o -> co (k o)", co=C))
                # global scale = mean |w|
                asum = small.tile([C, 1], F32, name=f"asum{j}")
                nc.vector.tensor_reduce(out=asum[:], in_=wflat[:],
                                        axis=mybir.AxisListType.X, op=ALU.add,
                                        apply_absolute_value=True)
                stot = small.tile([C, 1], F32, name=f"stot{j}")
                nc.gpsimd.partition_all_reduce(stot[:], asum[:], channels=C,
                                               reduce_op=bass_isa.ReduceOp.add)
                s_j = small.tile([C, 1], F32, name=f"s{j}")
                nc.vector.tensor_scalar_mul(s_j[:], stot[:], 1.0 / NW)
                sfac.append(s_j)
                # [co, ci, tap] -> tap-major copy -> 32x32 block transpose
                wtap = small.tile([C, 9, C], F32, name=f"wtap{j}")
                nc.vector.tensor_copy(
                    out=wtap[:],
                    in_=wflat.rearrange("p (ci t) -> p t ci", t=9))
                wT = small.tile([C, 9, C], F32, name=f"wT{j}")
                nc.vector.transpose(out=wT[:], in_=wtap[:])
                sw8 = small.tile([C, 9, C], F8, name=f"sw8_{j}")
                nc.scalar.sign(out=sw8[:], in_=wT[:])
                # block-diagonal stationaries, one per tap
                sts = []
                for t in range(9):
                    st = small.tile([P, P], F8, name=f"st{j}_{t}")
                    nc.gpsimd.memset(st[:], 0.0)
                    for g in range(IMG):
                        nc.gpsimd.dma_start(
                            out=st[g * C:(g + 1) * C, g * C:(g + 1) * C],
                            in_=sw8[:, t, :])
                    sts.append(st)
                st_tiles.append(sts)
                gs = small.tile([C, 1], F32, name=f"gs{j}")
                nc.sync.dma_start(out=gs[:], in_=g_in[j].unsqueeze(1))
                bs = small.tile([C, 1], F32, name=f"bs{j}")
                nc.sync.dma_start(out=bs[:], in_=be_in[j].unsqueeze(1))
                gsb.append(gs)
                bsb.append(bs)

            eps32 = small.tile([C, 1], F32)
            nc.vector.memset(eps32[:], EPS)

            # ---- load x and build B1 = sign(x), 16 chunks of 10 rows
            for c in range(16):
                xc = xsb[:, c * 1600:(c + 1) * 1600]
                nc.sync.dma_start(out=xc, in_=x2d[:, c * 1600:(c + 1) * 1600])
                nc.scalar.sign(
                    out=_valid_ap(B, PADH + (10 * c + 1) * Wp + 1, 10, Wp),
                    in_=xc.rearrange("p (r c) -> p r c", c=W))

            pmode = mybir.MatmulPerfMode.DoublePixel if USE_DP else None

            def conv(j):
                sts = st_tiles[j]
                for g0 in range(0, len(CHUNKS), 8):
                    grp = CHUNKS[g0:g0 + 8]
                    pts = [ps.tile([P, nr * Wp], F32, name="pt")
                           for (_, nr) in grp]
                    for t in range(9):
                        for pt, (r0, nr) in zip(pts, grp):
                            base = PADH + r0 * Wp + DELTA[t]
                            nc.tensor.matmul(
                                pt[:], sts[t][:],
                                B[:, base:base + nr * Wp],
                                start=(t == 0), stop=(t == 8),
                                perf_mode=pmode)
                    for k, (pt, (r0, nr)) in enumerate(zip(pts, grp)):
                        zflat = z[:, (r0 - 1) * W:(r0 - 1 + nr) * W]
                        zv = zflat.rearrange("p (r c) -> p r c", c=W)
                        nc.scalar.copy(out=zv, in_=_valid_ap(pt, 1, nr, Wp))
                        nc.vector.bn_stats(out=stats[:, g0 + k, :], in_=zflat)

            def finalize(j):
                """-> (a128, b32) per-channel affine: y_bn = a*z + b.

                stats[:, k, :] = (cntE, meanE, cntE*varE, cntO, meanO,
                cntO*varO) per chunk. Weighted-exact combine:
                S1 = sum cnt*mean; S2 = sum (cnt*var + cnt*mean^2).
                """
                def rsum(src, dst):
                    nc.vector.tensor_reduce(out=dst, in_=src,
                                            axis=mybir.AxisListType.X,
                                            op=ALU.add)

                acc = small.tile([P, 8], F32, name=f"acc{j}")
                # junk = cnt*mean (even/odd); acc cols = partial reductions
                nc.vector.tensor_mul(junk[:], scol(0), scol(1))
                rsum(junk[:], acc[:, 0:1])
                nc.vector.tensor_mul(junk[:], scol(3), scol(4))
                rsum(junk[:], acc[:, 1:2])
                # cnt*mean^2 terms and cnt*var terms for S2
                m2e = small.tile([P, NCH], F32, name=f"m2e{j}")
                nc.vector.tensor_mul(m2e[:], scol(1), scol(1))
                nc.vector.tensor_mul(junk[:], scol(0), m2e[:])
                rsum(junk[:], acc[:, 2:3])
                nc.vector.tensor_mul(m2e[:], scol(4), scol(4))
                nc.vector.tensor_mul(junk[:], scol(3), m2e[:])
                rsum(junk[:], acc[:, 3:4])
                rsum(scol(2), acc[:, 4:5])
                rsum(scol(5), acc[:, 5:6])
                s1 = small.tile([P, 1], F32, name=f"s1_{j}")
                nc.vector.tensor_add(s1[:], acc[:, 0:1], acc[:, 1:2])
                s2 = small.tile([P, 1], F32, name=f"s2_{j}")
                nc.vector.tensor_add(s2[:], acc[:, 2:3], acc[:, 3:4])
                nc.vector.tensor_add(s2[:], s2[:], acc[:, 4:5])
                nc.vector.tensor_add(s2[:], s2[:], acc[:, 5:6])
                ccin = dram.tile([P, 2], F32, name=f"ccin{j}")
                ccout = dram.tile([P, 2], F32, name=f"ccout{j}",
                                  addr_space="Shared")
                nc.sync.dma_start(out=ccin[:, 0:1], in_=s1[:])
                nc.sync.dma_start(out=ccin[:, 1:2], in_=s2[:])
                nc.gpsimd.collective_compute(
                    "AllReduce", ALU.add,
                    replica_groups=[list(range(N_CORES))],
                    ins=[ccin.opt()], outs=[ccout.opt()])
                gsum = small.tile([P, 2], F32, name=f"gsum{j}")
                nc.sync.dma_start(out=gsum[:], in_=ccout[:])
                gt = small.tile([C, 2, IMG], F32, name=f"gt{j}")
                for g in range(IMG):
                    nc.gpsimd.dma_start(out=gt[:, :, g:g + 1],
                                        in_=gsum[g * C:(g + 1) * C, :])
                cs = small.tile([C, 2], F32, name=f"cs{j}")
                nc.vector.tensor_reduce(out=cs[:], in_=gt[:],
                                        axis=mybir.AxisListType.X, op=ALU.add)
                nsamp = float(IMG * N_CORES * NPIX)
                m32 = small.tile([C, 1], F32, name=f"m32_{j}")
                nc.vector.tensor_scalar_mul(m32[:], cs[:, 0:1], 1.0 / nsamp)
                qb = small.tile([C, 1], F32, name=f"qb{j}")
                nc.vector.tensor_scalar_mul(qb[:], cs[:, 1:2], 1.0 / nsamp)
                msq = small.tile([C, 1], F32, name=f"msq{j}")
                nc.vector.tensor_mul(msq[:], m32[:], m32[:])
                v32 = small.tile([C, 1], F32, name=f"v32_{j}")
                nc.vector.tensor_tensor(out=v32[:], in0=qb[:], in1=msq[:],
                                        op=ALU.subtract)
                vy = small.tile([C, 1], F32, name=f"vy{j}")
                nc.vector.tensor_scalar(out=vy[:], in0=v32[:],
                                        scalar1=sfac[j][:], scalar2=sfac[j][:],
                                        op0=ALU.mult, op1=ALU.mult)
                sq = small.tile([C, 1], F32, name=f"sq{j}")
                nc.scalar.activation(out=sq[:], in_=vy[:], func=AF.Sqrt,
                                     bias=eps32[:], scale=1.0)
                r32 = small.tile([C, 1], F32, name=f"r32_{j}")
                nc.vector.reciprocal(out=r32[:], in_=sq[:])
                a32 = small.tile([C, 1], F32, name=f"a32_{j}")
                nc.vector.tensor_scalar(out=a32[:], in0=r32[:],
                                        scalar1=sfac[j][:], scalar2=gsb[j][:],
                                        op0=ALU.mult, op1=ALU.mult)
                negm = small.tile([C, 1], F32, name=f"negm{j}")
                nc.vector.tensor_scalar_mul(negm[:], m32[:], -1.0)
                b32 = small.tile([C, 1], F32, name=f"b32_{j}")
                nc.vector.scalar_tensor_tensor(
                    out=b32[:], in0=a32[:], scalar=negm[:], in1=bsb[j][:],
                    op0=ALU.mult, op1=ALU.add)
                a128 = small.tile([P, 1], F32, name=f"a128_{j}")
                for g in range(IMG):
                    nc.gpsimd.dma_start(out=a128[g * C:(g + 1) * C, :],
                                        in_=a32[:])
                return a128, b32

            def repl128(src, name):
                t = small.tile([P, 1], F32, name=name)
                for g in range(IMG):
                    nc.gpsimd.dma_start(out=t[g * C:(g + 1) * C, :],
                                        in_=src[:])
                return t

            # ===== conv1 =====
            conv(0)
            a1_128, b1_32 = finalize(0)
            b1_128 = repl128(b1_32, "b1_128")

            # ===== apply1: inner_nb = a1*z1 + x (in place); B2 = sign(.+b1)
            for c in range(16):
                xc = xsb[:, c * 1600:(c + 1) * 1600]
                zc = z[:, c * 1600:(c + 1) * 1600]
                nc.vector.scalar_tensor_tensor(
                    out=xc, in0=zc, scalar=a1_128[:], in1=xc,
                    op0=ALU.mult, op1=ALU.add)
                nc.scalar.activation(
                    out=_valid_ap(B, PADH + (10 * c + 1) * Wp + 1, 10, Wp),
                    in_=xc.rearrange("p (r c) -> p r c", c=W),
                    func=AF.Sign, bias=b1_128[:], scale=1.0)

            # ===== conv2 =====
            conv(1)
            a2_128, b2_32 = finalize(1)
            b12_32 = small.tile([C, 1], F32)
            nc.vector.tensor_add(b12_32[:], b1_32[:], b2_32[:])
            b12_128 = repl128(b12_32, "b12_128")

            # ===== final: out = a2*z2 + inner_nb + (b1+b2)
            for c in range(16):
                xc = xsb[:, c * 1600:(c + 1) * 1600]
                zc = z[:, c * 1600:(c + 1) * 1600]
                nc.vector.scalar_tensor_tensor(
                    out=xc, in0=zc, scalar=a2_128[:], in1=xc,
                    op0=ALU.mult, op1=ALU.add)
                nc.scalar.activation(out=xc, in_=xc, func=AF.Identity,
                                     bias=b12_128[:], scale=1.0)
                nc.sync.dma_start(out=out2d[:, c * 1600:(c + 1) * 1600],
                                  in_=xc)

    nc.compile()
    return nc


def kernel(x, w1, gamma1, beta1, w2, gamma2, beta2):
    if "nc" not in _CACHE:
        _CACHE["nc"] = _build()
    nc = _CACHE["nc"]
    x = np.ascontiguousarray(x, dtype=np.float32)
    in_maps = []
    for c in range(N_CORES):
        in_maps.append({
            "x": x[c * IMG:(c + 1) * IMG],
            "w1": np.ascontiguousarray(w1, np.float32),
            "gamma1": np.ascontiguousarray(gamma1, np.float32),
            "beta1": np.ascontiguousarray(beta1, np.float32),
            "w2": np.ascontiguousarray(w2, np.float32),
            "gamma2": np.ascontiguousarray(gamma2, np.float32),
            "beta2": np.ascontiguousarray(beta2, np.float32),
        })
    res = run_bass_kernel_spmd(nc, in_maps, list(range(N_CORES)))
    return np.concatenate([res.results[c]["out"] for c in range(N_CORES)],
                          axis=0)
